# revision 2
# baseline (speedup 1.0000x reference)
"""Trainium2 Bass kernel for nn_DecoderPolicyGradient (teacher-forced LSTM decoder).

Model: B=128, T=20, E=H=512, V=10000.
  xs[t] = features (t=0) | embed(captions[:, t-1])
  (h, c) = LSTM(xs[t], (h, c));  logits[t] = h @ W_lin.T + b_lin
  out = logits, time-major flattened: [T*B, V] fp32.

Sharding: pure data-parallel over batch, B/8 = 16 rows per NeuronCore, no
collectives. Per-core plan (everything "transposed": the 128-partition axis
carries hidden/gate dims and batch lives in the free dim):

  1. XgT[2048, 320] = W_ih @ xs.T + (b_ih + b_hh): one batched matmul over
     all 20 steps; the bias rides the ACT psum->SBUF copy.
  2. 20 serial LSTM steps at B=16: gatesT[2048, 16] = W_hh @ h + XgT[:, t]
     as 16 m-tiles of [128, 16] (FWL LDWEIGHTS+MATMUL pairs ~25-27 ns).
  3. logits[320, 10000] = H @ W_lin.T in bf16, staged into SBUF row-chunk
     buffers and written out with few LARGE contiguous DMAs (input loads on
     the SP HWDGE ring, output stores on the ACT HWDGE ring so they never
     queue behind each other). Row chunks: [0,128) avail@step8,
     [128,192) avail@12 (64 rows; pairs vocab-half slices vertically in
     psum so copies stay partition-aligned), [192,320) avail@20 (tail).

Host side does data movement only: embedding row gather, weight re-layouts
(m-major contiguous halves so weight DMAs use 8KB descriptors), final
bf16->f32 upcast + 8 x [320, 10000] -> [2560, 10000] reassembly.
"""

import sys

sys.path.insert(0, "/opt/trn_rl_repo")

from contextlib import ExitStack

import ml_dtypes
import numpy as np

import concourse.mybir as mybir
import concourse.tile as tile
from concourse import bacc
from concourse.bass_utils import run_bass_kernel_spmd

BF16 = mybir.dt.bfloat16
F32 = mybir.dt.float32
AF = mybir.ActivationFunctionType

B, T, E, H, V = 128, 20, 512, 512, 10000
NC = 8
BL = B // NC  # 16 batch rows per core
R = BL * T  # 320 output rows per core
KT = 4  # k-tiles of 128 over E/H
GT = 16  # m-tiles of 128 over 4H
VS = 512  # vocab n-slice width
HV = 5120  # vocab half split for chunk1 vertical pairing
N_SLICES = [(s, min(VS, V - s)) for s in range(0, V, VS)]  # 20 slices
UNITS_PER_STEP = 3

_cache = {}


def _build_nc(use_blin):
    nc = bacc.Bacc("TRN2", target_bir_lowering=False, debug=False)

    xsT_d = nc.dram_tensor("xsT", [128, KT, R], BF16, kind="ExternalInput").ap()
    wihT_d = nc.dram_tensor("wihT", [128, 2, KT, 8 * 128], BF16, kind="ExternalInput").ap()
    whhT_d = nc.dram_tensor("whhT", [128, 2, KT, 8 * 128], BF16, kind="ExternalInput").ap()
    bsum_d = nc.dram_tensor("bsum", [128, GT], F32, kind="ExternalInput").ap()
    wlinT_d = nc.dram_tensor("wlinT", [128, KT, V], BF16, kind="ExternalInput").ap()
    blin_d = nc.dram_tensor("blin", [1, V], BF16, kind="ExternalInput").ap()
    h0T_d = nc.dram_tensor("h0T", [128, KT, BL], BF16, kind="ExternalInput").ap()
    c0T_d = nc.dram_tensor("c0T", [128, KT, BL], F32, kind="ExternalInput").ap()
    out_d = nc.dram_tensor("out", [R, V], BF16, kind="ExternalOutput").ap()

    with tile.TileContext(nc) as tc, ExitStack() as ctx:
        const = ctx.enter_context(tc.tile_pool(name="const", bufs=1))
        work = ctx.enter_context(tc.tile_pool(name="work", bufs=2))
        psum_g = ctx.enter_context(tc.tile_pool(name="psum_g", bufs=1, space="PSUM"))
        psum_l = ctx.enter_context(tc.tile_pool(name="psum_l", bufs=2, space="PSUM"))

        # ---- persistent SBUF tensors
        xsT = const.tile([128, KT, R], BF16)
        wihT = const.tile([128, 2, KT, 8 * 128], BF16)
        whhT = const.tile([128, 2, KT, 8 * 128], BF16)
        bsum = const.tile([128, GT], F32)
        h0T = const.tile([128, KT, BL], BF16)
        c0T = const.tile([128, KT, BL], F32)
        blin = const.tile([1, V], BF16)
        ones = const.tile([1, 128], BF16)
        wlinT = const.tile([128, KT, V], BF16)
        xgT = const.tile([128, GT, R], BF16)
        hstore = const.tile([128, KT, R], BF16)
        obuf0 = const.tile([128, V], BF16)
        obuf1 = const.tile([128, HV], BF16)
        obuf2 = const.tile([128, V], BF16)

        # ---- input DMAs: one SP-HWDGE ring, FIFO = priority order.
        # Critical path first (xsT + wihT half0 -> phase-1 m0..7 can start),
        # wlinT (10 MB) last in 4 slabs so slab 0 lands well before step 8.
        nc.sync.dma_start(xsT[:], xsT_d[:])
        nc.sync.dma_start(bsum[:], bsum_d[:])
        nc.sync.dma_start(wihT[:, 0], wihT_d[:, 0])
        nc.sync.dma_start(wihT[:, 1], wihT_d[:, 1])
        nc.sync.dma_start(whhT[:, 0], whhT_d[:, 0])
        nc.sync.dma_start(whhT[:, 1], whhT_d[:, 1])
        nc.sync.dma_start(h0T[:], h0T_d[:])
        nc.sync.dma_start(c0T[:], c0T_d[:])
        if use_blin:
            nc.sync.dma_start(blin[:], blin_d[:])
            nc.gpsimd.memset(ones[:], 1.0)
        for s in range(0, V, 2560):
            w = min(2560, V - s)
            nc.sync.dma_start(wlinT[:, :, s : s + w], wlinT_d[:, :, s : s + w])

        # ---- phase 1: XgT[2048, R] = W_ih @ xs.T + bsum
        for m in range(GT):
            pxg = psum_l.tile([128, R], F32, tag=f"pl{m % 2}")
            for k in range(KT):
                nc.tensor.matmul(
                    pxg[:],
                    wihT[:, m // 8, k, (m % 8) * 128 : (m % 8 + 1) * 128],
                    xsT[:, k, :],
                    start=(k == 0),
                    stop=(k == KT - 1),
                )
            nc.scalar.activation(
                xgT[:, m, :], pxg[:], AF.Identity, bias=bsum[:, m : m + 1]
            )

        # ---- logits unit emitters (phase 3, interleaved into phase 2)
        copy_flip = [0]

        def mm_group(pl_ap, ms, rows, s, w):
            for k in range(KT):
                nc.tensor.matmul(
                    pl_ap,
                    hstore[:, k, ms : ms + rows],
                    wlinT[:, k, s : s + w],
                    start=(k == 0),
                    stop=(k == KT - 1) and not use_blin,
                )
            if use_blin:
                nc.tensor.matmul(
                    pl_ap, ones[:, :rows], blin[:, s : s + w],
                    start=False, stop=True,
                )

        def copy_out(dst_ap, src_ap):
            cp = nc.vector.tensor_copy if copy_flip[0] % 2 == 0 else nc.scalar.copy
            copy_flip[0] += 1
            cp(dst_ap, src_ap)

        def emit_unit(ci, j):
            if ci == 0 or ci == 2:
                ms = 0 if ci == 0 else 192
                ob = obuf0 if ci == 0 else obuf2
                s, w = N_SLICES[j]
                pl = psum_l.tile([128, VS], F32, tag=f"pl{j % 2}")
                mm_group(pl[:, :w], ms, 128, s, w)
                copy_out(ob[:, s : s + w], pl[:, :w])
            else:
                # chunk1: 64 rows (128..192); vertical pairing: psum top half
                # = vocab slice j (low half), bottom half = slice j+10.
                sl, wl = N_SLICES[j]
                sh, wh = N_SLICES[j + 10]
                pl = psum_l.tile([128, VS], F32, tag=f"pl{j % 2}")
                mm_group(pl[0:64, :wl], 128, 64, sl, wl)
                mm_group(pl[64:128, :wh], 128, 64, sh, wh)
                copy_out(obuf1[:, j * VS : j * VS + VS], pl[:])

        def fire_dma(ci, half):
            if ci == 0 or ci == 2:
                ms = 0 if ci == 0 else 192
                ob = obuf0 if ci == 0 else obuf2
                s0, s1 = (0, HV) if half == 0 else (HV, V)
                nc.scalar.dma_start(
                    out_d[ms : ms + 128, s0:s1], ob[:, s0:s1]
                )
            else:
                # both halves depend on all 10 paired units
                nc.scalar.dma_start(out_d[128:192, 0:HV], obuf1[0:64, :])
                nc.scalar.dma_start(out_d[128:192, HV:V], obuf1[64:128, 0 : V - HV])

        # unit queue: (chunk, j, first-step-whose-tail-can-host-it)
        queue = (
            [(0, j, 8) for j in range(20)]
            + [(1, j, 12) for j in range(10)]
            + [(2, j, 20) for j in range(20)]
        )
        done = {0: 0, 1: 0, 2: 0}

        def emit_and_track(ci, j):
            emit_unit(ci, j)
            done[ci] += 1
            if ci == 0 or ci == 2:
                if done[ci] == 10:
                    fire_dma(ci, 0)
                elif done[ci] == 20:
                    fire_dma(ci, 1)
            elif done[ci] == 10:
                fire_dma(ci, 0)

        # ---- phase 2: 20 serial LSTM steps (B = 16)
        cT_prev = c0T
        hT_prev = h0T
        qi = 0
        for t in range(T):
            lo = t * BL
            pg = []
            for g in range(4):
                p = psum_g.tile([128, 4, BL], F32, tag=f"pg{g}")
                pg.append(p)
                for mi in range(4):
                    m = g * 4 + mi
                    for k in range(KT):
                        nc.tensor.matmul(
                            p[:, mi, :],
                            whhT[:, m // 8, k, (m % 8) * 128 : (m % 8 + 1) * 128],
                            hT_prev[:, k, :],
                            start=(k == 0),
                            stop=(k == KT - 1),
                        )
            gates = work.tile([128, GT, BL], BF16, tag="gates")
            for g in range(4):
                nc.vector.tensor_add(
                    gates[:, 4 * g : 4 * g + 4, :],
                    pg[g][:],
                    xgT[:, 4 * g : 4 * g + 4, lo : lo + BL],
                )
            act_if = work.tile([128, 8, BL], BF16, tag="actif")
            act_g = work.tile([128, 4, BL], BF16, tag="actg")
            act_o = work.tile([128, 4, BL], BF16, tag="acto")
            nc.scalar.activation(act_if[:], gates[:, 0:8, :], AF.Sigmoid)
            nc.scalar.activation(act_g[:], gates[:, 8:12, :], AF.Tanh)
            nc.scalar.activation(act_o[:], gates[:, 12:16, :], AF.Sigmoid)
            ig = work.tile([128, 4, BL], F32, tag="ig")
            fc = work.tile([128, 4, BL], F32, tag="fc")
            nc.vector.tensor_mul(ig[:], act_if[:, 0:4, :], act_g[:])
            nc.vector.tensor_mul(fc[:], act_if[:, 4:8, :], cT_prev[:])
            c_new = work.tile([128, 4, BL], F32, tag="c")
            nc.vector.tensor_add(c_new[:], fc[:], ig[:])
            tc_b = work.tile([128, 4, BL], BF16, tag="tanhc")
            nc.scalar.activation(tc_b[:], c_new[:], AF.Tanh)
            h_new = hstore[:, :, lo : lo + BL]
            nc.vector.tensor_mul(h_new, act_o[:], tc_b[:])
            cT_prev = c_new
            hT_prev = h_new
            n_emit = 0
            while (
                qi < len(queue)
                and n_emit < UNITS_PER_STEP
                and queue[qi][2] <= t
            ):
                emit_and_track(queue[qi][0], queue[qi][1])
                qi += 1
                n_emit += 1

        while qi < len(queue):
            emit_and_track(queue[qi][0], queue[qi][1])
            qi += 1

    nc.compile()
    return nc


def _prep_inputs(features, captions, h0, c0, embed_w, W_ih, W_hh, b_ih, b_hh,
                 W_lin, b_lin):
    """Host-side layout prep (data movement only). Returns per-core in_maps."""
    bf = ml_dtypes.bfloat16
    f32 = np.float32

    features = np.asarray(features, f32)
    captions = np.asarray(captions)
    h0 = np.asarray(h0, f32)
    c0 = np.asarray(c0, f32)
    embed_w = np.asarray(embed_w, f32)
    W_ih = np.asarray(W_ih, f32)
    W_hh = np.asarray(W_hh, f32)
    b_ih = np.asarray(b_ih, f32)
    b_hh = np.asarray(b_hh, f32)
    W_lin = np.asarray(W_lin, f32)
    b_lin = np.asarray(b_lin, f32)

    # xs: [B, T, E] = [features, embed(captions[:, :T-1])]
    xs = np.empty((B, T, E), f32)
    xs[:, 0, :] = features
    xs[:, 1:, :] = embed_w[captions[:, : T - 1]]

    def to_kpm(w):  # [512, M] -> [128, KT, M] with row = k*128 + p
        return np.ascontiguousarray(w.reshape(KT, 128, w.shape[1]).transpose(1, 0, 2))

    def to_halves(w_kpm):  # [128, KT, 2048] -> [128, 2, KT, 1024] (m-major)
        return np.ascontiguousarray(
            w_kpm.reshape(128, KT, 2, 8 * 128).transpose(0, 2, 1, 3)
        )

    wihT = to_halves(to_kpm(W_ih.T)).astype(bf)
    whhT = to_halves(to_kpm(W_hh.T)).astype(bf)
    wlinT = to_kpm(W_lin.T).astype(bf)
    bsum = np.ascontiguousarray((b_ih + b_hh).reshape(GT, 128).T).astype(f32)
    blin = b_lin.reshape(1, V).astype(bf)

    in_maps = []
    for j in range(NC):
        sl = slice(j * BL, (j + 1) * BL)
        x = xs[sl]  # [BL, T, E]
        xsT = x.transpose(2, 1, 0).reshape(KT, 128, T * BL).transpose(1, 0, 2)
        h0T = h0[sl].T.reshape(KT, 128, BL).transpose(1, 0, 2)
        c0T = c0[sl].T.reshape(KT, 128, BL).transpose(1, 0, 2)
        in_maps.append(
            {
                "xsT": np.ascontiguousarray(xsT).astype(bf),
                "wihT": wihT,
                "whhT": whhT,
                "bsum": bsum,
                "wlinT": wlinT,
                "blin": blin,
                "h0T": np.ascontiguousarray(h0T).astype(bf),
                "c0T": np.ascontiguousarray(c0T).astype(f32),
            }
        )
    return in_maps


def kernel(**inputs) -> np.ndarray:
    maxlen = int(inputs.get("maxlen", T))
    assert maxlen == T, f"kernel hardcodes T={T}, got maxlen={maxlen}"
    use_blin = bool(np.any(np.asarray(inputs["b_lin"])))
    key = ("nc", use_blin)
    if key not in _cache:
        _cache[key] = _build_nc(use_blin)
    nc = _cache[key]
    in_maps = _prep_inputs(
        inputs["features"], inputs["captions"], inputs["h0"], inputs["c0"],
        inputs["embed_w"], inputs["W_ih"], inputs["W_hh"], inputs["b_ih"],
        inputs["b_hh"], inputs["W_lin"], inputs["b_lin"],
    )
    res = run_bass_kernel_spmd(nc, in_maps, list(range(NC)))
    # reassemble: core j rows (t*BL + b) -> full rows (t*B + j*BL + b)
    out = np.empty((T * B, V), np.float32)
    ov = out.reshape(T, NC, BL, V)
    for j in range(NC):
        ov[:, j] = res.results[j]["out"].reshape(T, BL, V).astype(np.float32)
    return out
